# revision 1
# baseline (speedup 1.0000x reference)
"""Trainium2 Bass kernel for fused MHA (B=4, T=2048, H=8, Hd=64, D=512).

Sharding: 8 cores = 4 batches x 2 head-groups (4 heads each).  Each core
computes QKV projection + attention for its (batch, head-group) slice and
writes a [T, 256] slice of the output; the host reassembles the full
[B, T, 512] tensor.

Per-core dataflow (all matmuls bf16 operands, fp32 PSUM accumulate):
  - host supplies x[b]^T cast to bf16 ([512, T]) and the per-group QKV
    weight columns (de-interleaved from the fused [512, 1536] W).
  - Q^T, K^T ([64, T] per head) via W-stationary matmuls.
  - V in natural [T, 260] layout (4 heads x 65 cols: 64 V columns + a ones
    column) via x^T-stationary matmuls; the ones columns make the PV matmul
    also produce the softmax denominator.
  - per head, per 1024-wide query block: S^T chunks [128, 1024] -> PSUM,
    exp via ScalarE (scale=1/8 folded into the activation), PV accumulate
    into PSUM [65, 1024] over the 16 key chunks.
  - PE-transpose [65, 128] blocks, reciprocal of the denominator column,
    normalize, DMA out.
"""

import sys

sys.path.insert(0, "/opt/trn_rl_repo")

import numpy as np
import ml_dtypes

import concourse.bass as bass
import concourse.mybir as mybir
import concourse.tile as tile
from concourse.bass_utils import run_bass_kernel_spmd
from concourse.masks import make_identity

import bass_rust

B, T, D = 4, 2048, 512
H, HD = 8, 64
HPG = 4  # heads per group (per core)
GC = HPG * HD  # output cols per core = 256
N_CORES = 8
KC = D // 128  # contraction chunks for the QKV projection
TKC = T // 128  # key chunks
NQB = 2  # query blocks per half (of 512)
QHALF = 1024  # query span per psum_o

BF16 = mybir.dt.bfloat16
F32 = mybir.dt.float32


def _fix_multiwait(nc):
    """walrus in this container accepts a single sync-wait per instruction;
    Tile emits several on some (the tail drain, multi-queue DMA consumers).
    Hoist extras onto same-engine NoOp carriers inserted just before."""
    for f in nc.m.functions:
        for bb in f.blocks:
            new_list = []
            changed = False
            for i in bb.instructions:
                si = i.sync_info
                if si is not None and len(si.on_wait) > 1:
                    waits = list(si.on_wait)
                    upd = list(si.on_update)
                    i.sync_info = bass_rust.SyncInfo(
                        on_wait=[waits[-1]], on_update=upd
                    )
                    for k, w in enumerate(waits[:-1]):
                        nop = mybir.InstNoOp(
                            name=f"{i.name}-waitsplit-{k}", ins=[], outs=[]
                        )
                        nop.engine = i.engine
                        nop.sync_info = bass_rust.SyncInfo(on_wait=[w], on_update=[])
                        new_list.append(nop)
                    changed = True
                new_list.append(i)
            if changed:
                bb.instructions = new_list
    return nc


def build_mha():
    nc = bass.Bass("TRN2", target_bir_lowering=False)
    xt = nc.dram_tensor("xt", [D, T], BF16, kind="ExternalInput")
    wq = nc.dram_tensor("wq", [D, GC], BF16, kind="ExternalInput")
    wk = nc.dram_tensor("wk", [D, GC], BF16, kind="ExternalInput")
    wv = nc.dram_tensor("wv", [D, GC], BF16, kind="ExternalInput")
    y = nc.dram_tensor("y", [T, GC], F32, kind="ExternalOutput")

    with tile.TileContext(nc) as tc:
        with (
            tc.tile_pool(name="persist", bufs=1) as pp,
            tc.tile_pool(name="work", bufs=3) as wp,
            tc.tile_pool(name="small", bufs=4) as sp,
            tc.tile_pool(name="ps", bufs=2, space="PSUM") as ps,
        ):
            # ---- loads ----
            xts = []
            for k in range(KC):
                t_ = pp.tile([128, T], BF16, tag=f"xt{k}")
                nc.sync.dma_start(t_[:], xt[k * 128 : (k + 1) * 128, :])
                xts.append(t_)
            wqs, wks, wvs = [], [], []
            for name, dram, lst in (("wq", wq, wqs), ("wk", wk, wks), ("wv", wv, wvs)):
                for k in range(KC):
                    t_ = pp.tile([128, GC], BF16, tag=f"{name}{k}")
                    nc.sync.dma_start(t_[:], dram[k * 128 : (k + 1) * 128, :])
                    lst.append(t_)

            ident = pp.tile([65, 65], F32, tag="ident")
            make_identity(nc, ident[:])

            # ---- QKV projection ----
            # Q^T / K^T: [128, T] tiles, two heads each (head 2m at rows 0:64,
            # head 2m+1 at rows 64:128)
            qts, kts = [], []
            copy_flip = 0
            for src, lst, nm in ((wqs, qts, "qt"), (wks, kts, "kt")):
                for m in range(2):
                    dst = pp.tile([128, T], BF16, tag=f"{nm}{m}")
                    for n in range(T // 512):
                        pq = ps.tile([128, 512], F32, tag="s")
                        for k in range(KC):
                            nc.tensor.matmul(
                                pq[:],
                                src[k][:, m * 128 : (m + 1) * 128],
                                xts[k][:, n * 512 : (n + 1) * 512],
                                start=(k == 0),
                                stop=(k == KC - 1),
                            )
                        out_ap = dst[:, n * 512 : (n + 1) * 512]
                        if copy_flip % 2 == 0:
                            nc.scalar.copy(out_ap, pq[:])
                        else:
                            nc.vector.tensor_copy(out_ap, pq[:])
                        copy_flip += 1
                    lst.append(dst)

            # V natural layout with interleaved ones columns: [128, 260]
            # per key chunk; per-head lhsT slice is cols h*65 .. h*65+64.
            vaug = []
            for t in range(TKC):
                vt = pp.tile([128, HPG * 65], BF16, tag=f"vaug{t}")
                # ones columns at 64, 129, 194, 259
                nc.gpsimd.memset(vt[:, 64 :: 65], 1.0)
                pv = ps.tile([128, GC], F32, tag="s")
                for k in range(KC):
                    nc.tensor.matmul(
                        pv[:],
                        xts[k][:, t * 128 : (t + 1) * 128],
                        wvs[k][:],
                        start=(k == 0),
                        stop=(k == KC - 1),
                    )
                # strided copy, skipping the ones columns
                src_ap = pv[:].rearrange("p (h d) -> p h d", h=HPG)
                dst_ap = vt[:].rearrange("p (h d) -> p h d", h=HPG)[:, :, 0:64]
                if copy_flip % 2 == 0:
                    nc.scalar.copy(dst_ap, src_ap)
                else:
                    nc.vector.tensor_copy(dst_ap, src_ap)
                copy_flip += 1
                vaug.append(vt)

            # ---- attention ----
            for h in range(HPG):
                m, p0 = h // 2, (h % 2) * 64
                qt_h = qts[m][p0 : p0 + 64, :]
                kt_h = kts[m][p0 : p0 + 64, :]
                for half in range(T // QHALF):
                    po = ps.tile([65, QHALF], F32, tag="o")
                    q0 = half * QHALF
                    for c in range(TKC):
                        pss = ps.tile([128, QHALF], F32, tag="s")
                        for qb in range(NQB):
                            nc.tensor.matmul(
                                pss[:, qb * 512 : (qb + 1) * 512],
                                kt_h[:, c * 128 : (c + 1) * 128],
                                qt_h[:, q0 + qb * 512 : q0 + (qb + 1) * 512],
                                start=True,
                                stop=True,
                            )
                        pb = wp.tile([128, QHALF], BF16, tag="pb")
                        nc.scalar.activation(
                            pb[:], pss[:], mybir.ActivationFunctionType.Exp,
                            scale=float(HD) ** -0.5,
                        )
                        for qb in range(NQB):
                            nc.tensor.matmul(
                                po[:, qb * 512 : (qb + 1) * 512],
                                vaug[c][:, h * 65 : h * 65 + 65],
                                pb[:, qb * 512 : (qb + 1) * 512],
                                start=(c == 0),
                                stop=(c == TKC - 1),
                            )
                    ot = wp.tile([65, QHALF], F32, tag="ot")
                    nc.vector.tensor_copy(ot[:], po[:])
                    for tq in range(QHALF // 128):
                        pt = ps.tile([128, 65], F32, tag="s")
                        nc.tensor.transpose(
                            pt[:], ot[:, tq * 128 : (tq + 1) * 128], ident[:]
                        )
                        rcp = sp.tile([128, 1], F32, tag="rcp")
                        nc.vector.reciprocal(rcp[:], pt[:, 64:65])
                        res = sp.tile([128, 64], F32, tag="res")
                        nc.vector.tensor_scalar_mul(res[:], pt[:, 0:64], rcp[:])
                        nc.sync.dma_start(
                            y[q0 + tq * 128 : q0 + (tq + 1) * 128,
                              h * 64 : (h + 1) * 64],
                            res[:],
                        )
    _fix_multiwait(nc)
    return nc


_CACHE = {}


def _get_nc():
    if "nc" not in _CACHE:
        _CACHE["nc"] = build_mha()
    return _CACHE["nc"]


def _split_w(W, b):
    """De-interleave the fused QKV weight: W[:, h*192 + 3*hd + c] is the
    (head h, dim hd) column of q/k/v for c=0/1/2 (torch reshape
    [B,T,H,Hd,3] with the size-3 axis innermost)."""
    hd = np.arange(HD)
    per = {}
    for g in range(2):
        cols_q, cols_k, cols_v = [], [], []
        for hl in range(HPG):
            h = g * HPG + hl
            base = h * (HD * 3)
            cols_q.append(base + 3 * hd + 0)
            cols_k.append(base + 3 * hd + 1)
            cols_v.append(base + 3 * hd + 2)
        per[g] = tuple(
            np.ascontiguousarray(W[:, np.concatenate(cs)]).astype(ml_dtypes.bfloat16)
            for cs in (cols_q, cols_k, cols_v)
        )
    return per


def kernel(x, mask, W, b):
    x = np.asarray(x, dtype=np.float32)
    mask = np.asarray(mask)
    W = np.asarray(W, dtype=np.float32)
    b = np.asarray(b, dtype=np.float32)

    if not np.all(mask == 1.0):
        return _fallback(x, mask, W, b)
    if b.any():
        return _fallback(x, mask, W, b)

    per_g = _split_w(W, b)
    in_maps = []
    for bi in range(B):
        xtb = np.ascontiguousarray(x[bi].T).astype(ml_dtypes.bfloat16)
        for g in range(2):
            wq_, wk_, wv_ = per_g[g]
            in_maps.append({"xt": xtb, "wq": wq_, "wk": wk_, "wv": wv_})

    nc = _get_nc()
    res = run_bass_kernel_spmd(nc, in_maps, core_ids=list(range(N_CORES)))

    out = np.empty((B, T, D), dtype=np.float32)
    for bi in range(B):
        for g in range(2):
            out[bi, :, g * GC : (g + 1) * GC] = res.results[bi * 2 + g]["y"]
    return out


def _fallback(x, mask, W, b):
    """Reference-exact numpy path for inputs the device kernel does not
    specialize for (non-trivial mask or bias). Not exercised by the
    benchmark inputs (mask is all-ones, b is zero)."""
    qkv = np.einsum("btd,de->bte", x, W) + b
    qkv = qkv.reshape(B, T, H, HD, 3).transpose(4, 0, 2, 1, 3)
    q, k, v = qkv[0], qkv[1], qkv[2]
    s = np.einsum("bhqd,bhkd->bhqk", q, k) / (HD**0.5)
    s = s + (1.0 - mask) * -10000.0
    s = s - s.max(-1, keepdims=True)
    e = np.exp(s)
    p = e / e.sum(-1, keepdims=True)
    o = np.einsum("bhqk,bhkd->bhqd", p, v)
    return o.transpose(0, 2, 1, 3).reshape(B, T, H * HD).astype(np.float32)


# revision 3
# speedup vs baseline: 1.0411x; 1.0411x over previous
"""Trainium2 Bass kernel for fused MHA (B=4, T=2048, H=8, Hd=64, D=512).

Sharding: 8 cores = 4 batches x 2 head-groups (4 heads each).  Each core
computes QKV projection + attention for its (batch, head-group) slice and
writes a [T, 256] slice of the output; the host reassembles the full
[B, T, 512] tensor.

Per-core dataflow (all matmuls bf16 operands, fp32 PSUM accumulate):
  - host supplies x[b]^T cast to bf16 ([512, T]) and the per-group QKV
    weight columns (de-interleaved from the fused [512, 1536] W).
  - Q^T, K^T ([64, T] per head) via W-stationary matmuls.
  - V in natural [T, 260] layout (4 heads x 65 cols: 64 V columns + a ones
    column) via x^T-stationary matmuls; the ones columns make the PV matmul
    also produce the softmax denominator.
  - per head, per 1024-wide query block: S^T chunks [128, 1024] -> PSUM,
    exp via ScalarE (scale=1/8 folded into the activation), PV accumulate
    into PSUM [65, 1024] over the 16 key chunks.
  - PE-transpose [65, 128] blocks, reciprocal of the denominator column,
    normalize, DMA out.
"""

import sys

sys.path.insert(0, "/opt/trn_rl_repo")

import numpy as np
import ml_dtypes

import concourse.bass as bass
import concourse.mybir as mybir
import concourse.tile as tile
from concourse.bass_utils import run_bass_kernel_spmd
from concourse.masks import make_identity

import bass_rust

B, T, D = 4, 2048, 512
H, HD = 8, 64
HPG = 4  # heads per group (per core)
GC = HPG * HD  # output cols per core = 256
N_CORES = 8
KC = D // 128  # contraction chunks for the QKV projection
TKC = T // 128  # key chunks
NQB = 2  # query blocks per half (of 512)
QHALF = 1024  # query span per psum_o

BF16 = mybir.dt.bfloat16
F32 = mybir.dt.float32


def _fix_multiwait(nc):
    """walrus in this container accepts a single sync-wait per instruction;
    Tile emits several on some (the tail drain, multi-queue DMA consumers).
    Hoist extras onto same-engine NoOp carriers inserted just before."""
    for f in nc.m.functions:
        for bb in f.blocks:
            new_list = []
            changed = False
            for i in bb.instructions:
                si = i.sync_info
                if si is not None and len(si.on_wait) > 1:
                    waits = list(si.on_wait)
                    upd = list(si.on_update)
                    i.sync_info = bass_rust.SyncInfo(
                        on_wait=[waits[-1]], on_update=upd
                    )
                    for k, w in enumerate(waits[:-1]):
                        nop = mybir.InstNoOp(
                            name=f"{i.name}-waitsplit-{k}", ins=[], outs=[]
                        )
                        nop.engine = i.engine
                        nop.sync_info = bass_rust.SyncInfo(on_wait=[w], on_update=[])
                        new_list.append(nop)
                    changed = True
                new_list.append(i)
            if changed:
                bb.instructions = new_list
    return nc


def build_mha():
    nc = bass.Bass("TRN2", target_bir_lowering=False)
    xt = nc.dram_tensor("xt", [D, T], BF16, kind="ExternalInput")
    wq = nc.dram_tensor("wq", [D, GC], BF16, kind="ExternalInput")
    wk = nc.dram_tensor("wk", [D, GC], BF16, kind="ExternalInput")
    wv = nc.dram_tensor("wv", [D, GC], BF16, kind="ExternalInput")
    y = nc.dram_tensor("y", [T, GC], F32, kind="ExternalOutput")

    with tile.TileContext(nc) as tc:
        with (
            tc.tile_pool(name="persist", bufs=1) as pp,
            tc.tile_pool(name="work", bufs=4) as wp,
            tc.tile_pool(name="small", bufs=4) as sp,
            tc.tile_pool(name="ps", bufs=1, space="PSUM") as ps,
        ):
            # ---- loads ----
            # x^T in [128, 512] chunk tiles so the QKV pipeline can start as
            # soon as the first column block lands.
            wqs, wks, wvs = [], [], []
            for name, dram, lst in (("wq", wq, wqs), ("wk", wk, wks), ("wv", wv, wvs)):
                for k in range(KC):
                    t_ = pp.tile([128, GC], BF16, tag=f"{name}{k}")
                    nc.sync.dma_start(t_[:], dram[k * 128 : (k + 1) * 128, :])
                    lst.append(t_)
            xts = [[None] * (T // 512) for _ in range(KC)]
            for n in range(T // 512):
                for k in range(KC):
                    t_ = pp.tile([128, 512], BF16, tag=f"xt{k}_{n}")
                    nc.sync.dma_start(
                        t_[:], xt[k * 128 : (k + 1) * 128, n * 512 : (n + 1) * 512]
                    )
                    xts[k][n] = t_

            ident = pp.tile([65, 65], F32, tag="ident")
            make_identity(nc, ident[:])

            # ---- QKV projection ----
            # Q^T / K^T: [128, T] tiles, two heads each (head 2m at rows 0:64,
            # head 2m+1 at rows 64:128).  V in natural layout with interleaved
            # ones columns ([128, 260] per key chunk; per-head lhsT slice is
            # cols h*65 .. h*65+64).  Grouped by x column block so each block
            # is fully consumed right after its DMA lands.
            qts = [pp.tile([128, T], BF16, tag=f"qt{m}", name=f"qt{m}") for m in range(2)]
            kts = [pp.tile([128, T], BF16, tag=f"kt{m}", name=f"kt{m}") for m in range(2)]
            vaug = []
            for t in range(TKC):
                vt = pp.tile([128, HPG * 65], BF16, tag=f"vaug{t}")
                nc.gpsimd.memset(vt[:, 64 :: 65], 1.0)  # ones at 64,129,194,259
                vaug.append(vt)
            copy_flip = 0
            for n in range(T // 512):
                for src, lst in ((wqs, qts), (wks, kts)):
                    for m in range(2):
                        pq = ps.tile([128, 512], F32, tag=f"s{copy_flip % 3}")
                        for k in range(KC):
                            nc.tensor.matmul(
                                pq[:],
                                src[k][:, m * 128 : (m + 1) * 128],
                                xts[k][n][:],
                                start=(k == 0),
                                stop=(k == KC - 1),
                            )
                        out_ap = lst[m][:, n * 512 : (n + 1) * 512]
                        if copy_flip % 2 == 0:
                            nc.scalar.copy(out_ap, pq[:])
                        else:
                            nc.vector.tensor_copy(out_ap, pq[:])
                        copy_flip += 1
                for t in range(4 * n, 4 * n + 4):
                    vt = vaug[t]
                    pv = ps.tile([128, GC], F32, tag=f"s{copy_flip % 3}")
                    for k in range(KC):
                        nc.tensor.matmul(
                            pv[:],
                            xts[k][n][:, (t % 4) * 128 : (t % 4 + 1) * 128],
                            wvs[k][:],
                            start=(k == 0),
                            stop=(k == KC - 1),
                        )
                    src_ap = pv[:].rearrange("p (h d) -> p h d", h=HPG)
                    dst_ap = vt[:].rearrange("p (h d) -> p h d", h=HPG)[:, :, 0:64]
                    if copy_flip % 2 == 0:
                        nc.scalar.copy(dst_ap, src_ap)
                    else:
                        nc.vector.tensor_copy(dst_ap, src_ap)
                    copy_flip += 1

            # ---- attention ----
            for h in range(HPG):
                m, p0 = h // 2, (h % 2) * 64
                qt_h = qts[m][p0 : p0 + 64, :]
                kt_h = kts[m][p0 : p0 + 64, :]
                for half in range(T // QHALF):
                    po = ps.tile([65, QHALF], F32, tag="o")
                    q0 = half * QHALF
                    for c in range(TKC):
                        pss = ps.tile([128, QHALF], F32, tag=f"s{c % 3}")
                        for qb in range(NQB):
                            nc.tensor.matmul(
                                pss[:, qb * 512 : (qb + 1) * 512],
                                kt_h[:, c * 128 : (c + 1) * 128],
                                qt_h[:, q0 + qb * 512 : q0 + (qb + 1) * 512],
                                start=True,
                                stop=True,
                            )
                        pb = wp.tile([128, QHALF], BF16, tag="pb")
                        nc.scalar.activation(
                            pb[:], pss[:], mybir.ActivationFunctionType.Exp,
                            scale=float(HD) ** -0.5,
                        )
                        for qb in range(NQB):
                            nc.tensor.matmul(
                                po[:, qb * 512 : (qb + 1) * 512],
                                vaug[c][:, h * 65 : h * 65 + 65],
                                pb[:, qb * 512 : (qb + 1) * 512],
                                start=(c == 0),
                                stop=(c == TKC - 1),
                            )
                    ot = wp.tile([65, QHALF], F32, tag="ot")
                    nc.vector.tensor_copy(ot[:], po[:])
                    for tq in range(QHALF // 128):
                        pt = ps.tile([128, 65], F32, tag=f"s{tq % 3}")
                        nc.tensor.transpose(
                            pt[:], ot[:, tq * 128 : (tq + 1) * 128], ident[:]
                        )
                        rcp = sp.tile([128, 1], F32, tag="rcp")
                        nc.vector.reciprocal(rcp[:], pt[:, 64:65])
                        res = sp.tile([128, 64], F32, tag="res")
                        nc.vector.tensor_scalar_mul(res[:], pt[:, 0:64], rcp[:])
                        nc.sync.dma_start(
                            y[q0 + tq * 128 : q0 + (tq + 1) * 128,
                              h * 64 : (h + 1) * 64],
                            res[:],
                        )
    _fix_multiwait(nc)
    return nc


_CACHE = {}


def _get_nc():
    if "nc" not in _CACHE:
        _CACHE["nc"] = build_mha()
    return _CACHE["nc"]


def _split_w(W, b):
    """De-interleave the fused QKV weight: W[:, h*192 + 3*hd + c] is the
    (head h, dim hd) column of q/k/v for c=0/1/2 (torch reshape
    [B,T,H,Hd,3] with the size-3 axis innermost)."""
    hd = np.arange(HD)
    per = {}
    for g in range(2):
        cols_q, cols_k, cols_v = [], [], []
        for hl in range(HPG):
            h = g * HPG + hl
            base = h * (HD * 3)
            cols_q.append(base + 3 * hd + 0)
            cols_k.append(base + 3 * hd + 1)
            cols_v.append(base + 3 * hd + 2)
        per[g] = tuple(
            np.ascontiguousarray(W[:, np.concatenate(cs)]).astype(ml_dtypes.bfloat16)
            for cs in (cols_q, cols_k, cols_v)
        )
    return per


def kernel(x, mask, W, b):
    x = np.asarray(x, dtype=np.float32)
    mask = np.asarray(mask)
    W = np.asarray(W, dtype=np.float32)
    b = np.asarray(b, dtype=np.float32)

    if not np.all(mask == 1.0):
        return _fallback(x, mask, W, b)
    if b.any():
        return _fallback(x, mask, W, b)

    per_g = _split_w(W, b)
    in_maps = []
    for bi in range(B):
        xtb = np.ascontiguousarray(x[bi].T).astype(ml_dtypes.bfloat16)
        for g in range(2):
            wq_, wk_, wv_ = per_g[g]
            in_maps.append({"xt": xtb, "wq": wq_, "wk": wk_, "wv": wv_})

    nc = _get_nc()
    res = run_bass_kernel_spmd(nc, in_maps, core_ids=list(range(N_CORES)))

    out = np.empty((B, T, D), dtype=np.float32)
    for bi in range(B):
        for g in range(2):
            out[bi, :, g * GC : (g + 1) * GC] = res.results[bi * 2 + g]["y"]
    return out


def _fallback(x, mask, W, b):
    """Reference-exact numpy path for inputs the device kernel does not
    specialize for (non-trivial mask or bias). Not exercised by the
    benchmark inputs (mask is all-ones, b is zero)."""
    qkv = np.einsum("btd,de->bte", x, W) + b
    qkv = qkv.reshape(B, T, H, HD, 3).transpose(4, 0, 2, 1, 3)
    q, k, v = qkv[0], qkv[1], qkv[2]
    s = np.einsum("bhqd,bhkd->bhqk", q, k) / (HD**0.5)
    s = s + (1.0 - mask) * -10000.0
    s = s - s.max(-1, keepdims=True)
    e = np.exp(s)
    p = e / e.sum(-1, keepdims=True)
    o = np.einsum("bhqk,bhkd->bhqd", p, v)
    return o.transpose(0, 2, 1, 3).reshape(B, T, H * HD).astype(np.float32)


# revision 5
# speedup vs baseline: 1.3335x; 1.2809x over previous
"""Trainium2 Bass kernel for fused MHA (B=4, T=2048, H=8, Hd=64, D=512).

Sharding: 8 cores = 4 batches x 2 head-groups (4 heads each).  Each core
computes QKV projection + attention for its (batch, head-group) slice and
writes a [T, 256] slice of the output; the host reassembles the full
[B, T, 512] tensor.

Per-core dataflow (all matmuls bf16 operands, fp32 PSUM accumulate):
  - host supplies x[b]^T cast to bf16 ([512, T]) and the per-group QKV
    weight columns (de-interleaved from the fused [512, 1536] W).
  - Q^T, K^T ([64, T] per head) via W-stationary matmuls.
  - V in natural [T, 260] layout (4 heads x 65 cols: 64 V columns + a ones
    column) via x^T-stationary matmuls; the ones columns make the PV matmul
    also produce the softmax denominator.
  - per head, per 1024-wide query block: S^T chunks [128, 1024] -> PSUM,
    exp via ScalarE (scale=1/8 folded into the activation), PV accumulate
    into PSUM [65, 1024] over the 16 key chunks.
  - PE-transpose [65, 128] blocks, reciprocal of the denominator column,
    normalize, DMA out.
"""

import sys

sys.path.insert(0, "/opt/trn_rl_repo")

import numpy as np
import ml_dtypes

import concourse.bass as bass
import concourse.mybir as mybir
import concourse.tile as tile
from concourse.bass_utils import run_bass_kernel_spmd
from concourse.masks import make_identity

import bass_rust

B, T, D = 4, 2048, 512
H, HD = 8, 64
HPG = 4  # heads per group (per core)
GC = HPG * HD  # output cols per core = 256
N_CORES = 8
KC = D // 128  # contraction chunks for the QKV projection
TKC = T // 128  # key chunks
NQB = 2  # query blocks per half (of 512)
QHALF = 1024  # query span per psum_o

BF16 = mybir.dt.bfloat16
F32 = mybir.dt.float32


def _fix_multiwait(nc):
    """walrus in this container accepts a single sync-wait per instruction;
    Tile emits several on some (the tail drain, multi-queue DMA consumers).
    Hoist extras onto same-engine NoOp carriers inserted just before."""
    for f in nc.m.functions:
        for bb in f.blocks:
            new_list = []
            changed = False
            for i in bb.instructions:
                si = i.sync_info
                if si is not None and len(si.on_wait) > 1:
                    waits = list(si.on_wait)
                    upd = list(si.on_update)
                    i.sync_info = bass_rust.SyncInfo(
                        on_wait=[waits[-1]], on_update=upd
                    )
                    for k, w in enumerate(waits[:-1]):
                        nop = mybir.InstNoOp(
                            name=f"{i.name}-waitsplit-{k}", ins=[], outs=[]
                        )
                        nop.engine = i.engine
                        nop.sync_info = bass_rust.SyncInfo(on_wait=[w], on_update=[])
                        new_list.append(nop)
                    changed = True
                new_list.append(i)
            if changed:
                bb.instructions = new_list
    return nc


def build_mha():
    nc = bass.Bass("TRN2", target_bir_lowering=False)
    xt = nc.dram_tensor("xt", [D, T], BF16, kind="ExternalInput")
    wq = nc.dram_tensor("wq", [D, GC], BF16, kind="ExternalInput")
    wk = nc.dram_tensor("wk", [D, GC], BF16, kind="ExternalInput")
    wv = nc.dram_tensor("wv", [D, GC], BF16, kind="ExternalInput")
    y = nc.dram_tensor("y", [T, GC], F32, kind="ExternalOutput")

    with tile.TileContext(nc) as tc:
        with (
            tc.tile_pool(name="persist", bufs=1) as pp,
            tc.tile_pool(name="work", bufs=4) as wp,
            tc.tile_pool(name="small", bufs=4) as sp,
            tc.tile_pool(name="ps", bufs=1, space="PSUM") as ps,
        ):
            # ---- loads ----
            # x^T in [128, 512] chunk tiles so the QKV pipeline can start as
            # soon as the first column block lands.
            wqs, wks, wvs = [], [], []
            for name, dram, lst in (("wq", wq, wqs), ("wk", wk, wks), ("wv", wv, wvs)):
                t_ = pp.tile([128, KC * GC], BF16, tag=f"{name}_all", name=f"{name}_all")
                nc.sync.dma_start(
                    t_[:].rearrange("p (k c) -> p k c", k=KC),
                    dram[:].rearrange("(k p) c -> p k c", p=128),
                )
                lst.extend(t_[:, k * GC : (k + 1) * GC] for k in range(KC))
            xtn = []
            for n in range(T // 512):
                t_ = pp.tile([128, KC * 512], BF16, tag=f"xtn{n}", name=f"xtn{n}")
                nc.scalar.dma_start(
                    t_[:].rearrange("p (k c) -> p k c", k=KC),
                    xt[:, n * 512 : (n + 1) * 512].rearrange(
                        "(k p) c -> p k c", p=128
                    ),
                )
                xtn.append(t_)
            xts = [
                [xtn[n][:, k * 512 : (k + 1) * 512] for n in range(T // 512)]
                for k in range(KC)
            ]

            ident = pp.tile([65, 65], F32, tag="ident")
            make_identity(nc, ident[:])

            # ---- QKV projection ----
            # Q^T / K^T: [128, T] tiles, two heads each (head 2m at rows 0:64,
            # head 2m+1 at rows 64:128).  V in natural layout with interleaved
            # ones columns ([128, 260] per key chunk; per-head lhsT slice is
            # cols h*65 .. h*65+64).  Grouped by x column block so each block
            # is fully consumed right after its DMA lands.
            qts = [pp.tile([128, T], BF16, tag=f"qt{m}", name=f"qt{m}") for m in range(2)]
            kts = [pp.tile([128, T], BF16, tag=f"kt{m}", name=f"kt{m}") for m in range(2)]
            vaug = []
            for t in range(TKC):
                vt = pp.tile([128, HPG * 65], BF16, tag=f"vaug{t}")
                nc.gpsimd.memset(vt[:, 64 :: 65], 1.0)  # ones at 64,129,194,259
                vaug.append(vt)
            copy_flip = 0
            for n in range(T // 512):
                for src, lst in ((wqs, qts), (wks, kts)):
                    for m in range(2):
                        pq = ps.tile([128, 512], F32, tag=f"s{copy_flip % 3}")
                        for k in range(KC):
                            nc.tensor.matmul(
                                pq[:],
                                src[k][:, m * 128 : (m + 1) * 128],
                                xts[k][n],
                                start=(k == 0),
                                stop=(k == KC - 1),
                            )
                        out_ap = lst[m][:, n * 512 : (n + 1) * 512]
                        if copy_flip % 2 == 0:
                            nc.scalar.copy(out_ap, pq[:])
                        else:
                            nc.vector.tensor_copy(out_ap, pq[:])
                        copy_flip += 1
                for t in range(4 * n, 4 * n + 4):
                    vt = vaug[t]
                    pv = ps.tile([128, GC], F32, tag=f"s{copy_flip % 3}")
                    for k in range(KC):
                        nc.tensor.matmul(
                            pv[:],
                            xts[k][n][:, (t % 4) * 128 : (t % 4 + 1) * 128],
                            wvs[k],
                            start=(k == 0),
                            stop=(k == KC - 1),
                        )
                    src_ap = pv[:].rearrange("p (h d) -> p h d", h=HPG)
                    dst_ap = vt[:].rearrange("p (h d) -> p h d", h=HPG)[:, :, 0:64]
                    if copy_flip % 2 == 0:
                        nc.scalar.copy(dst_ap, src_ap)
                    else:
                        nc.vector.tensor_copy(dst_ap, src_ap)
                    copy_flip += 1

            # ---- attention ----
            for h in range(HPG):
                m, p0 = h // 2, (h % 2) * 64
                qt_h = qts[m][p0 : p0 + 64, :]
                kt_h = kts[m][p0 : p0 + 64, :]
                for half in range(T // QHALF):
                    po = ps.tile([65, QHALF], F32, tag="o")
                    q0 = half * QHALF
                    for c in range(TKC):
                        pss = ps.tile([128, QHALF], F32, tag=f"s{c % 3}")
                        for qb in range(NQB):
                            nc.tensor.matmul(
                                pss[:, qb * 512 : (qb + 1) * 512],
                                kt_h[:, c * 128 : (c + 1) * 128],
                                qt_h[:, q0 + qb * 512 : q0 + (qb + 1) * 512],
                                start=True,
                                stop=True,
                            )
                        pb = wp.tile([128, QHALF], BF16, tag="pb")
                        nc.scalar.activation(
                            pb[:], pss[:], mybir.ActivationFunctionType.Exp,
                            scale=float(HD) ** -0.5,
                        )
                        for qb in range(NQB):
                            nc.tensor.matmul(
                                po[:, qb * 512 : (qb + 1) * 512],
                                vaug[c][:, h * 65 : h * 65 + 65],
                                pb[:, qb * 512 : (qb + 1) * 512],
                                start=(c == 0),
                                stop=(c == TKC - 1),
                            )
                    ot = wp.tile([65, QHALF], F32, tag="ot")
                    nc.vector.tensor_copy(ot[:], po[:])
                    for tq in range(QHALF // 128):
                        pt = ps.tile([128, 65], F32, tag="o")
                        nc.tensor.transpose(
                            pt[:], ot[:, tq * 128 : (tq + 1) * 128], ident[:]
                        )
                        rcp = sp.tile([128, 1], F32, tag="rcp")
                        nc.vector.reciprocal(rcp[:], pt[:, 64:65])
                        res = sp.tile([128, 64], F32, tag="res")
                        nc.vector.tensor_scalar_mul(res[:], pt[:, 0:64], rcp[:])
                        nc.sync.dma_start(
                            y[q0 + tq * 128 : q0 + (tq + 1) * 128,
                              h * 64 : (h + 1) * 64],
                            res[:],
                        )
    _fix_multiwait(nc)
    return nc


_CACHE = {}


def _get_nc():
    if "nc" not in _CACHE:
        _CACHE["nc"] = build_mha()
    return _CACHE["nc"]


def _split_w(W, b):
    """De-interleave the fused QKV weight: W[:, h*192 + 3*hd + c] is the
    (head h, dim hd) column of q/k/v for c=0/1/2 (torch reshape
    [B,T,H,Hd,3] with the size-3 axis innermost)."""
    hd = np.arange(HD)
    per = {}
    for g in range(2):
        cols_q, cols_k, cols_v = [], [], []
        for hl in range(HPG):
            h = g * HPG + hl
            base = h * (HD * 3)
            cols_q.append(base + 3 * hd + 0)
            cols_k.append(base + 3 * hd + 1)
            cols_v.append(base + 3 * hd + 2)
        per[g] = tuple(
            np.ascontiguousarray(W[:, np.concatenate(cs)]).astype(ml_dtypes.bfloat16)
            for cs in (cols_q, cols_k, cols_v)
        )
    return per


def kernel(x, mask, W, b):
    x = np.asarray(x, dtype=np.float32)
    mask = np.asarray(mask)
    W = np.asarray(W, dtype=np.float32)
    b = np.asarray(b, dtype=np.float32)

    if not np.all(mask == 1.0):
        return _fallback(x, mask, W, b)
    if b.any():
        return _fallback(x, mask, W, b)

    per_g = _split_w(W, b)
    in_maps = []
    for bi in range(B):
        xtb = np.ascontiguousarray(x[bi].T).astype(ml_dtypes.bfloat16)
        for g in range(2):
            wq_, wk_, wv_ = per_g[g]
            in_maps.append({"xt": xtb, "wq": wq_, "wk": wk_, "wv": wv_})

    nc = _get_nc()
    res = run_bass_kernel_spmd(nc, in_maps, core_ids=list(range(N_CORES)))

    out = np.empty((B, T, D), dtype=np.float32)
    for bi in range(B):
        for g in range(2):
            out[bi, :, g * GC : (g + 1) * GC] = res.results[bi * 2 + g]["y"]
    return out


def _fallback(x, mask, W, b):
    """Reference-exact numpy path for inputs the device kernel does not
    specialize for (non-trivial mask or bias). Not exercised by the
    benchmark inputs (mask is all-ones, b is zero)."""
    qkv = np.einsum("btd,de->bte", x, W) + b
    qkv = qkv.reshape(B, T, H, HD, 3).transpose(4, 0, 2, 1, 3)
    q, k, v = qkv[0], qkv[1], qkv[2]
    s = np.einsum("bhqd,bhkd->bhqk", q, k) / (HD**0.5)
    s = s + (1.0 - mask) * -10000.0
    s = s - s.max(-1, keepdims=True)
    e = np.exp(s)
    p = e / e.sum(-1, keepdims=True)
    o = np.einsum("bhqk,bhkd->bhqd", p, v)
    return o.transpose(0, 2, 1, 3).reshape(B, T, H * HD).astype(np.float32)
